# revision 18
# baseline (speedup 1.0000x reference)
"""MLA (multi-head latent attention) prefill kernel for 8 TRN2 NeuronCores.

Sharding: tensor-parallel over heads (2 heads/core) for q / kv_b / attention /
wo; data-parallel over the sequence for the kv_a latent projection (each core
computes its 512 positions; the rms-normalized latent + roped k_pe are
AllGathered).  q is computed directly per-core for its own 2 heads over the
FULL sequence (same total flops as all-heads-on-a-shard, but no AllToAll and
no q staging round-trip).  Each core produces a partial wo output [D, S] in
bf16; the host sums the 8 partials.

Device layout notes (matmuls bf16, fp32 PSUM):
 - x is transposed on the host; projections run channel-major with weights
   stationary, x moving (free dim 512).
 - Per-head qk channel order is [rope_lo(32); rope_hi(32); nope(64)], rope
   pairs deinterleaved on the host.  cos/sin are staged duplicated as
   [64, 512] tiles so RoPE is 2 multiplies + add + sub on DVE per q block.
 - rmsnorm: the 1/rms scale g is folded into the latent BEFORE the gather
   (g broadcast across partitions with one fp32 ones-outer-product matmul),
   so there is no g collective and no per-block k/v rescale.
 - Scores are k-major: st[k, q] = k_tile^T q, two k-tiles per PSUM group
   ([128, 2, 512] = 2 banks) so exp runs once per pair on the ACT engine
   (amortizes the ~290ns ACT instruction overhead).
 - Causal masking: fully-masked (k,q) blocks are skipped; boundary tiles are
   computed unmasked and the upper triangle of probs is zeroed with one DVE
   affine_select per pair (exp(-1e9) == 0 in the reference as well).  A
   general additive mask falls back to identity-matmul accumulation of mask
   tiles into the score PSUM (baseline scheme).
 - Denominators: ones-column matmul per k-tile accumulated into a [1, 512]
   PSUM row that shares the attention-output PSUM tile (bank 1); the
   reciprocal (DVE 1-op approx, ~1e-5 rel) is broadcast across partitions
   with an ones-row fp32 matmul into the same bank, and one DVE multiply
   writes the normalized attention output to attn_T.
 - wo consumes attn_T [128, 2, S]; outputs are staged as [128, 2, 512] PSUM
   pairs, copied to bf16 on DVE and DMAd as [D, S] bf16 partials (halves the
   output write traffic).  The host sums partials in fp32 and transposes.
"""

import os
import sys

sys.path.insert(0, "/opt/trn_rl_repo")

import numpy as np
import ml_dtypes

import concourse.bass as bass
import concourse.tile as tile
import concourse.mybir as mybir
from concourse import bacc
from concourse.bass_utils import run_bass_kernel_spmd
from concourse.masks import make_identity

BF16 = mybir.dt.bfloat16
F32 = mybir.dt.float32
NPBF16 = ml_dtypes.bfloat16

S = 4096          # sequence length
D = 2048          # model dim
H = 16            # total heads
HPC = 2           # heads per core
NCORES = 8
L = 1024          # kv lora rank
LH = L // 2
ROPE = 64
NOPE = 64
VH = 128          # v head dim
SCALE = 128.0 ** -0.5
EPS = 1e-6

SB = 512          # free-dim block size
NSB = S // SB     # 8
NE = D // 128     # 16 e-chunks
NL = L // 128     # 8 latent chunks
NKT = S // 128    # 32 k tiles

last_results = None   # BassKernelResults of the most recent run (for test.py)

_BUILD_CACHE: dict = {}


def _build(skip, add, causal):
    """Build + schedule the per-core Bass program.

    skip/add: [NKT][NSB] bool grids over (k-tile, q-block) mask blocks.
    causal: True when the add-blocks are exactly the standard causal
    boundary tiles (mask applied by zeroing probs via affine_select).
    """
    nc = bacc.Bacc("TRN2", target_bir_lowering=False, debug=False,
                   num_devices=NCORES)

    groups = [list(range(NCORES))]
    need_mask = (not causal) and bool(np.asarray(add).any())

    wqT_d = nc.dram_tensor("wqT", [D, 128 * HPC], BF16, kind="ExternalInput")
    wkvaT_d = nc.dram_tensor("wkvaT", [D, 128 + ROPE], BF16, kind="ExternalInput")
    wkvbk_d = nc.dram_tensor("wkvbTk", [L, NOPE * HPC], BF16, kind="ExternalInput")
    wkvbv_d = nc.dram_tensor("wkvbTv", [L, VH * HPC], BF16, kind="ExternalInput")
    woT_d = nc.dram_tensor("woT", [VH * HPC, D], BF16, kind="ExternalInput")
    xT_d = nc.dram_tensor("xT", [D, S], BF16, kind="ExternalInput")
    xS_d = nc.dram_tensor("xS", [D, SB], BF16, kind="ExternalInput")
    cosT_d = nc.dram_tensor("cosT", [32, S], F32, kind="ExternalInput")
    sinT_d = nc.dram_tensor("sinT", [32, S], F32, kind="ExternalInput")
    cosS_d = nc.dram_tensor("cosS", [32, SB], F32, kind="ExternalInput")
    sinS_d = nc.dram_tensor("sinS", [32, SB], F32, kind="ExternalInput")
    if need_mask:
        maskT_d = nc.dram_tensor("maskT", [S, S], BF16, kind="ExternalInput")
    out_d = nc.dram_tensor("out", [D, S], BF16, kind="ExternalOutput")

    xT_r = xT_d[:].rearrange("(eo p) s -> p eo s", p=128)
    xS_r = xS_d[:].rearrange("(eo p) s -> p eo s", p=128)
    wq_r = wqT_d[:].rearrange("(eo p) c -> p eo c", p=128)
    wkva_r = wkvaT_d[:].rearrange("(eo p) c -> p eo c", p=128)
    out_r = out_d[:].rearrange("(mo p) s -> p mo s", p=128)

    # kv_a is L-sharded: core c computes latent channels [128c, 128c+128) for
    # every position, one s-block at a time; each s-block is gathered (lat
    # slice + a bf16 row of per-core sum-of-squares partials) as soon as it
    # is computed, so the collectives pipeline with compute instead of being
    # a startup barrier.  One gather per s-block (129 rows) rather than per
    # pair halves the latency until the first kvb/attn can run.
    latsb_sh = [nc.dram_tensor(f"latsb_sh{i}", [129, SB], BF16)
                for i in range(NSB)]
    latsb_full = [nc.dram_tensor(f"latsb_full{i}", [NCORES, 129, SB], BF16,
                                 addr_space="Shared") for i in range(NSB)]
    kpe_sh_d = nc.dram_tensor("kpe_sh", [ROPE, SB], BF16)
    kpe_full = nc.dram_tensor("kpe_full", [NCORES, ROPE, SB], BF16,
                              addr_space="Shared")

    # attention work order: q-block qb can run once kvb blocks 0..needed[qb]
    # are in; causal => needed[qb] == qb.
    needed = []
    for qb in range(NSB):
        act = [ki for ki in range(NKT) if not skip[ki][qb]]
        needed.append((max(act) // (SB // 128)) if act else -1)

    from contextlib import ExitStack
    with tile.TileContext(nc) as tc:
        with (
            tc.tile_pool(name="singles", bufs=1) as singles,
            tc.tile_pool(name="persist", bufs=1) as persist,
        ):
            ones_c = singles.tile([128, 1], BF16)
            nc.vector.memset(ones_c[:], 1.0)
            eps_t = singles.tile([1, 1], F32)
            nc.vector.memset(eps_t[:], EPS)
            if need_mask:
                ident = singles.tile([128, 128], BF16)
                make_identity(nc, ident[:])
            if causal:
                # 0/1 multiplicative boundary masks for diagonal tile pairs
                # (jj0 = 0 covers tiles jj 0,1; jj0 = 2 covers jj 2,3):
                # keep probs[p, j, f] where f - p - 128*(jj0+j) >= 0.
                mask01 = {}
                for jj0 in (0, 2):
                    mt_ = singles.tile([128, 2, SB], BF16, tag=f"mask01_{jj0}")
                    nc.gpsimd.memset(mt_[:], 1.0)
                    nc.gpsimd.affine_select(
                        out=mt_[:], in_=mt_[:],
                        pattern=[[-128, 2], [1, SB]],
                        compare_op=mybir.AluOpType.is_ge,
                        fill=0.0, base=-128 * jj0,
                        channel_multiplier=-1)
                    mask01[jj0] = mt_

            ones8 = singles.tile([8, 1], BF16)
            nc.vector.memset(ones8[:], 1.0)
            ones_rowb = singles.tile([1, 128], BF16)
            nc.vector.memset(ones_rowb[:], 1.0)
            wq_s = singles.tile([128, NE, 128 * HPC], BF16)
            wkva_s = singles.tile([128, NE, 128 + ROPE], BF16)
            wkvbk_s = singles.tile([128, NL, NOPE * HPC], BF16)
            wkvbv_s = singles.tile([128, NL, VH * HPC], BF16)
            wo_s = singles.tile([128, HPC, D], BF16)

            k0 = persist.tile([128, S], BF16)
            k1 = persist.tile([128, S], BF16)
            v_sb = persist.tile([128, NKT, VH * HPC], BF16)  # s-major v
            q_all = persist.tile([128, HPC, S], BF16)
            attn_T = persist.tile([128, HPC, S], BF16)

            with (
                tc.tile_pool(name="xp", bufs=2) as xp,
                tc.tile_pool(name="csp", bufs=2) as csp,
                tc.tile_pool(name="rp", bufs=1) as rp,
                tc.tile_pool(name="smal", bufs=1) as smal,
                tc.tile_pool(name="lg2", bufs=2) as lg2,
                tc.tile_pool(name="pb", bufs=1) as pb,
                tc.tile_pool(name="dvp", bufs=2) as dvp,
                tc.tile_pool(name="ost", bufs=2) as ost,
                tc.tile_pool(name="mp", bufs=1) as mp,
                tc.tile_pool(name="psS", bufs=2, space="PSUM") as psS,
                tc.tile_pool(name="psAD", bufs=2, space="PSUM") as psAD,
            ):
                # kpe inputs first: own x shard + small kv_a weight slice
                xs_t = xp.tile([128, NE, SB], BF16, tag="x")
                for e in range(NE):
                    nc.sync.dma_start(wkva_s[:, e, :], wkva_r[:, e, :])
                    nc.sync.dma_start(xs_t[:, e, :], xS_r[:, e, :])
                cs_o = csp.tile([64, SB], BF16, tag="cos")
                sn_o = csp.tile([64, SB], BF16, tag="sin")
                nc.gpsimd.dma_start(cs_o[0:32, :], cosS_d[:])
                nc.gpsimd.dma_start(cs_o[32:64, :], cosS_d[:])
                nc.gpsimd.dma_start(sn_o[0:32, :], sinS_d[:])
                nc.gpsimd.dma_start(sn_o[32:64, :], sinS_d[:])

                def rope(ps, cs, sn, out_lo, out_hi):
                    # ps: PSUM with rows [rope_lo(32); rope_hi(32)]
                    # m12 = [lo*cos; hi*cos]; m34 = [hi*sin; lo*sin] (swapped
                    # halves so the final sub/add see equal SBUF base
                    # partitions; PSUM inputs are exempt from that rule).
                    m12 = rp.tile([64, SB], BF16, tag="m12")
                    m34 = rp.tile([64, SB], BF16, tag="m34")
                    nc.vector.tensor_mul(m12[:], ps[0:64, :], cs[:])
                    nc.vector.tensor_mul(m34[0:32, :], ps[32:64, :], sn[0:32, :])
                    nc.vector.tensor_mul(m34[32:64, :], ps[0:32, :], sn[32:64, :])
                    nc.vector.tensor_sub(out_lo, m12[0:32, :], m34[0:32, :])
                    nc.vector.tensor_add(out_hi, m34[32:64, :], m12[32:64, :])

                # k_pe for the own s-block (tiny) -> smallest gather first
                kp_t = psS.tile([128, 2, SB], F32, tag="ps")
                for e in range(NE):
                    nc.tensor.matmul(kp_t[0:64, 0, :], wkva_s[:, e, 128:128 + ROPE],
                                     xs_t[:, e, :], start=(e == 0), stop=(e == NE - 1))
                kpe_t = smal.tile([64, SB], BF16, tag="kpe")
                rope(kp_t[0:64, 0, :], cs_o, sn_o, kpe_t[0:32, :], kpe_t[32:64, :])
                nc.sync.dma_start(kpe_sh_d[:], kpe_t[:])
                nc.gpsimd.collective_compute(
                    "AllGather", mybir.AluOpType.bypass, replica_groups=groups,
                    ins=[kpe_sh_d[:]], outs=[kpe_full[:]])

                def lat_block(sb, x_t):
                    # latent slice (own 128 channels) + sq partial, gathered
                    lp = psS.tile([128, 2, SB], F32, tag="ps")
                    for e in range(NE):
                        nc.tensor.matmul(lp[:, 0, :], wkva_s[:, e, 0:128],
                                         x_t[:, e, :],
                                         start=(e == 0), stop=(e == NE - 1))
                    # square on DVE (not ACT) so the ACT exp/ln tables are
                    # never evicted mid-kernel; square the SBUF bf16 copy so
                    # DVE never touches the PSUM bank the ACT copy is reading
                    lat_sb = smal.tile([128, SB], BF16, tag="latsb")
                    nc.scalar.copy(lat_sb[:], lp[:, 0, :])
                    sq_t = smal.tile([128, SB], BF16, tag="sq")
                    nc.vector.tensor_mul(sq_t[:], lat_sb[:], lat_sb[:])
                    nc.tensor.matmul(lp[0:1, 1, :], ones_c[:], sq_t[:],
                                     start=True, stop=True)
                    sqb = smal.tile([1, SB], BF16, tag="sqb")
                    nc.vector.tensor_copy(sqb[:], lp[0:1, 1, :])
                    nc.sync.dma_start(latsb_sh[sb][0:128, :], lat_sb[:])
                    nc.sync.dma_start(latsb_sh[sb][128:129, :], sqb[:])
                    nc.gpsimd.collective_compute(
                        "AllGather", mybir.AluOpType.bypass,
                        replica_groups=groups,
                        ins=[latsb_sh[sb][:]], outs=[latsb_full[sb][:]])

                def qproj(sb, x_t):
                    ssl = slice(sb * SB, (sb + 1) * SB)
                    cs = csp.tile([64, SB], BF16, tag="cos")
                    sn = csp.tile([64, SB], BF16, tag="sin")
                    nc.gpsimd.dma_start(cs[0:32, :], cosT_d[:, ssl])
                    nc.gpsimd.dma_start(cs[32:64, :], cosT_d[:, ssl])
                    nc.gpsimd.dma_start(sn[0:32, :], sinT_d[:, ssl])
                    nc.gpsimd.dma_start(sn[32:64, :], sinT_d[:, ssl])
                    for ct in range(HPC):
                        qp = psS.tile([128, 2, SB], F32, tag="ps")
                        for e in range(NE):
                            nc.tensor.matmul(qp[:, 0, :],
                                             wq_s[:, e, ct * 128:(ct + 1) * 128],
                                             x_t[:, e, :],
                                             start=(e == 0), stop=(e == NE - 1))
                        rope(qp[0:64, 0, :], cs, sn,
                             q_all[0:32, ct, ssl], q_all[32:64, ct, ssl])
                        nc.scalar.copy(q_all[64:128, ct, ssl], qp[64:128, 0, :])

                lg_cache = {}

                def kvb_load(sb):
                    ssl = slice(sb * SB, (sb + 1) * SB)
                    lg_t = lg2.tile([128, NL, SB], BF16, tag="lat")
                    nc.sync.dma_start(
                        lg_t[:],
                        latsb_full[sb][:, 0:128, :].rearrange("c p s -> p c s"))
                    sqp = smal.tile([8, SB], BF16, tag="sqp")
                    nc.sync.dma_start(sqp[:], latsb_full[sb][:, 128, :])
                    # g = 1/sqrt(mean(lat^2) + eps) from the 8 partials,
                    # broadcast across partitions, folded into the latent.
                    # Uses psS (immediately-consumed) tiles so psAD stays
                    # exclusive to attn's ad tiles (cross-block fin deferral
                    # relies on psAD cycling exactly with the head loop).
                    gp1 = psS.tile([128, 2, SB], F32, tag="ps")
                    nc.tensor.matmul(gp1[0:1, 0, :], ones8[:], sqp[:],
                                     start=True, stop=True)
                    # g = (mean + eps)^-0.5 via ln+exp: both live in the
                    # same ACT table set as the attention exp, so no
                    # ACT_TABLE_LOAD thrash mid-stream.
                    rs_t = smal.tile([1, SB], F32, tag="rs")
                    nc.scalar.activation(rs_t[:], gp1[0:1, 0, :],
                                         mybir.ActivationFunctionType.Ln,
                                         bias=eps_t[:], scale=1.0 / L)
                    g_sb = smal.tile([1, SB], BF16, tag="gsb")
                    nc.scalar.activation(g_sb[:], rs_t[:],
                                         mybir.ActivationFunctionType.Exp,
                                         scale=-0.5)
                    gp2 = psS.tile([128, 2, SB], F32, tag="ps")
                    nc.tensor.matmul(gp2[:, 1, :], ones_rowb[:], g_sb[:],
                                     start=True, stop=True)
                    for lt in range(NL):
                        nc.vector.tensor_mul(lg_t[:, lt, :], lg_t[:, lt, :],
                                             gp2[:, 1, :])
                    nc.sync.dma_start(k0[0:64, ssl], kpe_full[sb, :, :])
                    nc.vector.tensor_copy(k1[0:64, ssl], k0[0:64, ssl])
                    lg_cache[sb] = lg_t

                def kvb(sb):
                    ssl = slice(sb * SB, (sb + 1) * SB)
                    if sb not in lg_cache:
                        kvb_load(sb)
                    lg_t = lg_cache.pop(sb)

                    kbp = psS.tile([128, 2, SB], F32, tag="ps")
                    for lt in range(NL):
                        nc.tensor.matmul(kbp[:, 0, :], wkvbk_s[:, lt, :],
                                         lg_t[:, lt, :],
                                         start=(lt == 0), stop=(lt == NL - 1))
                    nc.vector.tensor_copy(k0[64:128, ssl], kbp[0:64, 0, :])
                    nc.vector.tensor_copy(k1[64:128, ssl], kbp[64:128, 0, :])

                    for st in range(SB // 128):
                        vp = psS.tile([128, 2, SB], F32, tag="ps")
                        for lt in range(NL):
                            nc.tensor.matmul(vp[:, 0, 0:VH * HPC],
                                             lg_t[:, lt, st * 128:(st + 1) * 128],
                                             wkvbv_s[:, lt, :],
                                             start=(lt == 0), stop=(lt == NL - 1))
                        nc.vector.tensor_copy(v_sb[:, sb * 4 + st, :],
                                              vp[:, 0, 0:VH * HPC])

                # fins (final 1/den normalize of a head-block) are deferred
                # ACROSS blocks: they pop at the start of the next attn()'s
                # head loops, so the PE never waits on the recip chain; wo
                # of block qb is likewise delayed until after attn(qb+1).
                pending = []

                def attn(qb):
                    qsl = slice(qb * SB, (qb + 1) * SB)
                    active = [ki for ki in range(NKT) if not skip[ki][qb]]
                    pairs = [active[i:i + 2] for i in range(0, len(active), 2)]
                    for h in range(HPC):
                        if pending:
                            pending.pop(0)()
                        kh = k0 if h == 0 else k1
                        ad = psAD.tile([128, 2, SB], F32, tag="ad")
                        # probs for the whole (h, qb) stay resident; the
                        # denominator is computed from PAIR SUMS built on the
                        # (otherwise idle) GpSimd engine, halving the number
                        # of ones-matmuls and keeping them off the exp chain.
                        probs = pb.tile([128, NKT, SB], BF16, tag="probs")
                        ps2 = pb.tile([128, 4, SB], BF16, tag="ps2")
                        npair = len(pairs)
                        flushed = 0

                        def av(pi):
                            pr = pairs[pi]
                            for j, ki in enumerate(pr):
                                nc.tensor.matmul(
                                    ad[:, 0, :],
                                    v_sb[:, ki, h * VH:(h + 1) * VH],
                                    probs[:, 2 * pi + j, :],
                                    start=(pi == 0 and j == 0),
                                    stop=(pi == npair - 1 and j == len(pr) - 1))

                        for pi, pr in enumerate(pairs):
                            sp2 = psS.tile([128, 2, SB], F32, tag="ps")
                            for j, ki in enumerate(pr):
                                has_m = need_mask and add[ki][qb]
                                nc.tensor.matmul(sp2[:, j, :],
                                                 kh[:, ki * 128:(ki + 1) * 128],
                                                 q_all[:, h, qsl],
                                                 start=True, stop=not has_m)
                                if has_m:
                                    m_t = mp.tile([128, SB], BF16, tag="mask")
                                    nc.sync.dma_start(
                                        m_t[:],
                                        maskT_d[ki * 128:(ki + 1) * 128, qsl])
                                    nc.tensor.matmul(sp2[:, j, :], ident[:],
                                                     m_t[:], start=False,
                                                     stop=True)
                            if pi >= 1:
                                av(pi - 1)
                            np_ = len(pr)
                            nc.scalar.activation(
                                probs[:, 2 * pi:2 * pi + np_, :], sp2[:, 0:np_, :],
                                mybir.ActivationFunctionType.Exp, scale=SCALE)
                            if pi % 4 == 0 and pi > 0:
                                # flush the PREVIOUS (complete) ps2 wave; its
                                # pair sums are long done, so no PE stall
                                for w in range(4):
                                    nc.tensor.matmul(
                                        ad[0:1, 1, :], ones_c[:], ps2[:, w, :],
                                        start=(flushed == 0 and w == 0),
                                        stop=False)
                                flushed += 4
                            if causal and add[pr[0]][qb]:
                                jj0 = pr[0] - 4 * qb
                                nc.vector.tensor_mul(
                                    probs[:, 2 * pi:2 * pi + np_, :],
                                    probs[:, 2 * pi:2 * pi + np_, :],
                                    mask01[jj0][:, 0:np_, :])
                            if np_ == 2:
                                nc.gpsimd.tensor_add(
                                    ps2[:, pi % 4, :], probs[:, 2 * pi, :],
                                    probs[:, 2 * pi + 1, :])
                            else:
                                nc.gpsimd.tensor_copy(
                                    ps2[:, pi % 4, :], probs[:, 2 * pi, :])
                        av(npair - 1)
                        for w in range(npair - flushed):
                            nc.tensor.matmul(ad[0:1, 1, :], ones_c[:],
                                             ps2[:, (flushed + w) % 4, :],
                                             start=(flushed == 0 and w == 0),
                                             stop=(w == npair - flushed - 1))
                        dv = dvp.tile([1, SB], F32, tag="dv")
                        nc.vector.reciprocal_approx_fast(out=dv[:], in_=ad[0:1, 1, :])
                        dvb = dvp.tile([1, SB], BF16, tag="dvb")
                        nc.vector.tensor_copy(dvb[:], dv[:])

                        def fin(ad=ad, dvb=dvb, h=h):
                            nc.tensor.matmul(ad[:, 1, :], ones_rowb[:], dvb[:],
                                             start=True, stop=True)
                            dbc = dvp.tile([128, SB], BF16, tag="dbc")
                            nc.vector.tensor_copy(dbc[:], ad[:, 1, :])
                            nc.vector.tensor_mul(attn_T[:, h, qsl], ad[:, 0, :],
                                                 dbc[:])

                        pending.append(fin)

                def wo(s2):
                    ssl = slice(s2 * SB, (s2 + 1) * SB)
                    for mpair in range(D // 256):
                        wp = psS.tile([128, 2, SB], F32, tag="ps")
                        for j in range(2):
                            mt = 2 * mpair + j
                            for cc in range(HPC):
                                nc.tensor.matmul(
                                    wp[:, j, :],
                                    wo_s[:, cc, mt * 128:(mt + 1) * 128],
                                    attn_T[:, cc, ssl],
                                    start=(cc == 0), stop=(cc == HPC - 1))
                        o_t = ost.tile([128, 2, SB], BF16, tag="o")
                        nc.vector.tensor_copy(o_t[:, 0, :], wp[:, 0, :])
                        nc.scalar.copy(o_t[:, 1, :], wp[:, 1, :])
                        nc.sync.dma_start(
                            out_r[:, 2 * mpair:2 * mpair + 2, ssl], o_t[:])

                # ---- main loop: lat/qproj stream, with kvb/attn/wo running
                #      PIPE blocks behind so each gather has ~2 blocks of
                #      compute time to land before its consumer issues ------
                PIPE = 3
                done_attn = [False] * NSB
                done_kvb = [False] * NSB
                woq = []   # attn'ed blocks whose wo hasn't issued yet

                def consume(sb):
                    if not done_kvb[sb]:
                        kvb(sb)
                        done_kvb[sb] = True
                    for qb in range(NSB):
                        if not done_attn[qb] and 0 <= needed[qb] <= sb:
                            attn(qb)
                            done_attn[qb] = True
                            woq.append(qb)
                            while len(woq) > 1:
                                wo(woq.pop(0))

                for sb in range(NSB):
                    ssl = slice(sb * SB, (sb + 1) * SB)
                    x_t = xp.tile([128, NE, SB], BF16, tag="x")
                    for e in range(NE):
                        nc.sync.dma_start(x_t[:, e, :], xT_r[:, e, ssl])
                    if sb == 0:
                        # late weight loads, split so they interleave with
                        # the x stream on the DMA queues
                        for e in range(NE):
                            nc.sync.dma_start(wq_s[:, e, :], wq_r[:, e, :])
                    if sb == 1:
                        nc.sync.dma_start(
                            wkvbk_s[:],
                            wkvbk_d[:].rearrange("(lo p) c -> p lo c", p=128))
                        nc.sync.dma_start(
                            wkvbv_s[:],
                            wkvbv_d[:].rearrange("(lo p) c -> p lo c", p=128))
                        nc.sync.dma_start(
                            wo_s[:], woT_d[:].rearrange("(co p) m -> p co m", p=128))
                    lat_block(sb, x_t)
                    qproj(sb, x_t)
                    if sb >= PIPE - 1 and sb - PIPE + 1 < NSB:
                        kvb_load(sb - PIPE + 1)
                    if sb >= PIPE:
                        consume(sb - PIPE)
                for sb in range(NSB - PIPE, NSB):
                    consume(sb)
                for qb in range(NSB):
                    if not done_attn[qb]:
                        if needed[qb] < 0:
                            for f in pending:
                                f()
                            pending.clear()
                            while woq:
                                wo(woq.pop(0))
                            nc.vector.memset(attn_T[:, :, qb * SB:(qb + 1) * SB], 0.0)
                        else:
                            attn(qb)
                        done_attn[qb] = True
                        woq.append(qb)
                        while len(woq) > 1:
                            wo(woq.pop(0))
                # flush the last block's deferred fins, then its wo
                for f in pending:
                    f()
                pending.clear()
                while woq:
                    wo(woq.pop(0))

    nc.compile()
    return nc, need_mask


def _causal_check(mask):
    """True when mask is exactly the standard causal mask (0 / <=-1e8)."""
    if mask.shape != (S, S):
        return False
    neg = mask <= -1e8
    zero = mask == 0.0
    if not np.all(neg | zero):
        return False
    expect = np.triu(np.ones((S, S), dtype=bool), k=1)
    return bool(np.array_equal(neg, expect))


def kernel(x, cos, sin, mask, wq, wkv_a, kv_norm_w, wkv_b, wo, start_pos=0):
    x = np.asarray(x, np.float32)
    cos = np.asarray(cos, np.float32)
    sin = np.asarray(sin, np.float32)
    mask = np.asarray(mask, np.float32)
    wq = np.asarray(wq, np.float32)
    wkv_a = np.asarray(wkv_a, np.float32)
    kv_norm_w = np.asarray(kv_norm_w, np.float32)
    wkv_b = np.asarray(wkv_b, np.float32)
    wo = np.asarray(wo, np.float32)

    # mask block metadata: [qb, qi, kt, kj]
    mr = mask.reshape(NSB, SB, NKT, 128)
    skip_qk = (mr <= -1e8).all(axis=(1, 3))          # [qb, kt]
    nonzero_qk = (mr != 0).any(axis=(1, 3))          # [qb, kt]
    skip = skip_qk.T.copy()                          # [kt, qb]
    add = (nonzero_qk & ~skip_qk).T.copy()
    causal = _causal_check(mask)
    key = (bool(causal), skip.tobytes(), add.tobytes())
    if key not in _BUILD_CACHE:
        _BUILD_CACHE[key] = _build(skip, add, causal)
    nc, need_mask = _BUILD_CACHE[key]

    # ---- host-side shard prep ----
    deint = np.concatenate([np.arange(0, ROPE, 2), np.arange(1, ROPE, 2)])
    wq_h = wq.reshape(H, 128, D)
    # per-head row order [rope deinterleaved; nope]
    qrows = np.concatenate([wq_h[:, NOPE + deint, :], wq_h[:, 0:NOPE, :]], axis=1)
    wkva_perm = np.concatenate([wkv_a[0:L], wkv_a[L + deint]], axis=0)
    wkvb_h = wkv_b.reshape(H, NOPE + VH, L)

    xT = np.ascontiguousarray(x[0].T).astype(NPBF16)
    cosT = np.ascontiguousarray(cos.T)
    sinT = np.ascontiguousarray(sin.T)
    shared = {"xT": xT, "cosT": cosT, "sinT": sinT}
    if need_mask:
        shared["maskT"] = np.ascontiguousarray(mask.T * (1.0 / SCALE)).astype(NPBF16)

    in_maps = []
    for c in range(NCORES):
        hs = [HPC * c + i for i in range(HPC)]
        k_rows = (wkvb_h[hs, 0:NOPE, :] * kv_norm_w[None, None, :]).reshape(
            NOPE * HPC, L)
        wkvbTk_c = np.ascontiguousarray(k_rows.T).astype(NPBF16)
        v_rows = wkvb_h[hs, NOPE:, :].reshape(VH * HPC, L)
        wkvbTv_c = np.ascontiguousarray(v_rows.T).astype(NPBF16)
        woT_c = np.ascontiguousarray(
            wo[:, hs[0] * VH:(hs[-1] + 1) * VH].T).astype(NPBF16)
        m = dict(shared)
        m.update({"wkvbTk": wkvbTk_c, "wkvbTv": wkvbTv_c, "woT": woT_c})
        wkva_rows = np.concatenate(
            [wkva_perm[128 * c:128 * (c + 1)], wkva_perm[L:]], axis=0)
        m["wkvaT"] = np.ascontiguousarray(wkva_rows.T).astype(NPBF16)
        ssl = slice(c * SB, (c + 1) * SB)
        m["xS"] = np.ascontiguousarray(xT[:, ssl])
        m["cosS"] = np.ascontiguousarray(cosT[:, ssl])
        m["sinS"] = np.ascontiguousarray(sinT[:, ssl])
        m["wqT"] = np.ascontiguousarray(
            qrows[hs].reshape(128 * HPC, D).T).astype(NPBF16)
        in_maps.append(m)

    trace = os.environ.get("KERNEL_TRACE", "0") == "1"
    if trace:
        _install_ntff_hook()
    global last_results
    last_results = run_bass_kernel_spmd(nc, in_maps, core_ids=list(range(NCORES)),
                                        trace=trace)
    total = np.zeros((D, S), np.float32)
    for r in last_results.results:
        total += np.asarray(r["out"], NPBF16).astype(np.float32)
    return np.ascontiguousarray(total.T)[None]


def _install_ntff_hook():
    """Register the axon NTFF profiling hook (used when KERNEL_TRACE=1)."""
    import types
    import ctypes
    import contextlib

    if "antenv.axon_hooks" in sys.modules:
        return
    try:
        so = ctypes.CDLL("/opt/axon/libaxon_pjrt.so")
        so.axon_start_nrt_profile
    except (OSError, AttributeError):
        return
    so.axon_start_nrt_profile.argtypes = [ctypes.POINTER(ctypes.c_int64),
                                          ctypes.c_size_t]
    so.axon_start_nrt_profile.restype = ctypes.c_int64
    so.axon_stop_nrt_profile.argtypes = [ctypes.c_char_p]
    so.axon_stop_nrt_profile.restype = ctypes.c_int64

    @contextlib.contextmanager
    def _hook(output_dir, device_ids):
        import jax
        jax.devices()
        if device_ids:
            ids = (ctypes.c_int64 * len(device_ids))(*device_ids)
            rc = so.axon_start_nrt_profile(ids, len(device_ids))
        else:
            rc = so.axon_start_nrt_profile(None, 0)
        if rc != 0:
            raise RuntimeError(f"axon_start_nrt_profile rc={rc}")
        try:
            yield
        finally:
            n = so.axon_stop_nrt_profile(str(output_dir).encode())
            if n < 0:
                raise RuntimeError(f"axon_stop_nrt_profile rc={n}")

    mod = types.ModuleType("antenv.axon_hooks")
    mod.get_axon_ntff_profile_hook = lambda: _hook
    mod.set_axon_ntff_profile_hook = lambda h: None
    sys.modules["antenv.axon_hooks"] = mod

